# revision 2
# baseline (speedup 1.0000x reference)
"""Trainium2 Bass kernel for a 6-layer transformer encoder (v2).

nn_Encoder: B=8, S=2048, D=512, NHEAD=8, D_FF=2048.

Strategy
--------
Data-parallel: one batch element per NeuronCore, no collectives.
The reference's reshape-without-transpose makes attention block-diagonal
over 8 slabs of 256 tokens; each slab's (256 x 512) block self-attends
as a (2048 x 64) matrix. Rows are permuted to (j*256 + s_local) order so
every operand is a natural slice of transposed activations.

v2 changes vs v1 (empirically driven, HW-measured):
- All matmuls in bf16: fp32r weight loads can't use Fast-Weight-Load and
  stream at ~0.7 ns/col on HW; bf16 runs at ~0.4 ns/col (N=512).
- All matmuls N=512 (PSUM-bank limit) instead of 256: amortizes
  LDWEIGHTS + fixed issue cost.
- Consecutive matmuls never target the same PSUM bank (same-bank
  back-to-back costs ~70ns); accumulation chains are emitted as
  interleaved pairs.
- exp() consumes [128,1024] PSUM mega-tiles spanning both attention
  parities -> one ACT instruction per (s-half, t-chunk).
- FFN/projection/wo matmuls of neighboring slab-pairs are interleaved
  into the ACT-bound attention inner loop (stealing), merging the
  previously separate attention/FFN phases.
- Weights are converted to bf16 on the host and DMA'd at half size.
"""

import numpy as np

P = 128
D = 512
S = 2048
FF = 2048
NH = 8          # slabs
T = 256         # tokens per slab
DH = 64
G = D // P      # 4
GF = FF // P    # 16
B = 8
EPS = 1e-5
N_LAYERS = 6
NPAIR = 4       # slab pairs
PT = 2 * T      # tokens per pair (512)

_CACHE = {}


def _build(n_layers=N_LAYERS, n_pairs=NPAIR):
    from collections import deque
    import concourse.bass as bass
    import concourse.tile as tile
    from concourse import bacc, mybir

    CDT = mybir.dt.bfloat16
    F32 = mybir.dt.float32
    U32 = mybir.dt.uint32
    AF = mybir.ActivationFunctionType
    OP = mybir.AluOpType

    nc = bacc.Bacc("TRN2", target_bir_lowering=False)

    xT_d = nc.dram_tensor("xT", (D, S), CDT, kind="ExternalInput")
    wq_d = nc.dram_tensor("wq", (N_LAYERS, D, D), CDT, kind="ExternalInput")
    wk_d = nc.dram_tensor("wk", (N_LAYERS, D, D), CDT, kind="ExternalInput")
    wv_d = nc.dram_tensor("wv", (N_LAYERS, D, D), CDT, kind="ExternalInput")
    wo_d = nc.dram_tensor("wo", (N_LAYERS, D, D), CDT, kind="ExternalInput")
    w1_d = nc.dram_tensor("w1", (N_LAYERS, D, FF), CDT, kind="ExternalInput")
    w2_d = nc.dram_tensor("w2", (N_LAYERS, FF, D), CDT, kind="ExternalInput")
    b1_d = nc.dram_tensor("b1", (N_LAYERS, FF), F32, kind="ExternalInput")
    b2_d = nc.dram_tensor("b2", (N_LAYERS, D), CDT, kind="ExternalInput")
    g1_d = nc.dram_tensor("g1", (N_LAYERS, D), F32, kind="ExternalInput")
    bt1_d = nc.dram_tensor("beta1", (N_LAYERS, D), F32, kind="ExternalInput")
    g2_d = nc.dram_tensor("g2", (N_LAYERS, D), F32, kind="ExternalInput")
    bt2_d = nc.dram_tensor("beta2", (N_LAYERS, D), F32, kind="ExternalInput")
    out_d = nc.dram_tensor("out", (D, S), CDT, kind="ExternalOutput")

    QO, KO, VO, OO = 0, D, 2 * D, 3 * D

    with tile.TileContext(nc) as tc:
        with tc.tile_pool(name="const", bufs=1) as cpool, \
             tc.tile_pool(name="x", bufs=1) as xpool, \
             tc.tile_pool(name="wa", bufs=1) as wpool, \
             tc.tile_pool(name="w1", bufs=1) as w1pool, \
             tc.tile_pool(name="w2", bufs=1) as w2pool, \
             tc.tile_pool(name="par", bufs=2) as parpool, \
             tc.tile_pool(name="qkv", bufs=2) as qkvpool, \
             tc.tile_pool(name="oT", bufs=2) as oTpool, \
             tc.tile_pool(name="fT", bufs=2) as fTpool, \
             tc.tile_pool(name="r", bufs=2) as rpool, \
             tc.tile_pool(name="pX", bufs=3) as pXpool, \
             tc.tile_pool(name="sm", bufs=2) as small, \
             tc.tile_pool(name="um", bufs=2, space="PSUM") as umpool, \
             tc.tile_pool(name="po", bufs=1, space="PSUM") as popool, \
             tc.tile_pool(name="pp", bufs=2, space="PSUM") as pppool:

            # ---------------- constants ----------------
            ones_f = cpool.tile([P, 16], F32, tag="ones_f")
            nc.vector.memset(ones_f, 1.0)
            ones_c = cpool.tile([P, 16], CDT, tag="ones_c")
            nc.vector.tensor_copy(out=ones_c, in_=ones_f)
            ones1 = cpool.tile([P, 1], CDT, tag="ones1")      # LN-stats lhsT
            nc.vector.tensor_copy(out=ones1, in_=ones_f[:, 0:1])
            ones_row = cpool.tile([1, PT], CDT, tag="ones_row")  # bias-fold rhs
            nc.vector.tensor_copy(out=ones_row,
                                  in_=ones_f[0:1, 0:1].to_broadcast((1, PT)))

            # persistent activations, transposed: xT_s[p, g, s] = x[s, g*128+p]
            xT_s = xpool.tile([P, G, S], CDT, tag="xT")
            for g in range(G):
                nc.sync.dma_start(xT_s[:, g, :], xT_d[g * P:(g + 1) * P, :])

            def quake_rsqrt(s1, s2, s3):
                """s3 holds var+eps on entry; writes rsqrt(s3) into s1.
                All [1, PT] f32 slices. Quake-III seed + 2 Newton steps,
                entirely on DVE (keeps ACT exp/relu-only)."""
                nc.vector.tensor_copy(out=s2, in_=s3.bitcast(U32))
                nc.vector.tensor_scalar(out=s2, in0=s2,
                                        scalar1=-0.5, scalar2=float(0x5F3759DF),
                                        op0=OP.mult, op1=OP.add)
                nc.vector.tensor_copy(out=s2.bitcast(U32), in_=s2)
                nc.vector.tensor_tensor(s1, s2, s2, OP.mult)
                nc.vector.tensor_tensor(s1, s3, s1, OP.mult)
                nc.vector.tensor_scalar(out=s1, in0=s1, scalar1=-0.5, scalar2=1.5,
                                        op0=OP.mult, op1=OP.add)
                nc.vector.tensor_tensor(s2, s2, s1, OP.mult)
                nc.vector.tensor_tensor(s1, s2, s2, OP.mult)
                nc.vector.tensor_tensor(s1, s3, s1, OP.mult)
                nc.vector.tensor_scalar(out=s1, in0=s1, scalar1=-0.5, scalar2=1.5,
                                        op0=OP.mult, op1=OP.add)
                nc.vector.tensor_tensor(s1, s2, s1, OP.mult)

            for l in range(n_layers):
                # -------- layer weights / params (single-buffered) --------
                wa = wpool.tile([P, G, 4 * D], CDT, tag="wa", name=f"wa_{l}")
                for wi, w_d in enumerate((wq_d, wk_d, wv_d, wo_d)):
                    for ki in range(G):
                        nc.sync.dma_start(wa[:, ki, wi * D:(wi + 1) * D],
                                          w_d[l, ki * P:(ki + 1) * P, :])
                w1s = w1pool.tile([P, G, FF], CDT, tag="w1s", name=f"w1s_{l}")
                for ki in range(G):
                    nc.sync.dma_start(w1s[:, ki, :], w1_d[l, ki * P:(ki + 1) * P, :])
                w2s = w2pool.tile([P, GF, D], CDT, tag="w2s", name=f"w2s_{l}")
                for fi in range(GF):
                    nc.sync.dma_start(w2s[:, fi, :], w2_d[l, fi * P:(fi + 1) * P, :])
                g1s = parpool.tile([P, G], F32, tag="g1", name=f"g1_{l}")
                bt1s = parpool.tile([P, G], F32, tag="bt1", name=f"bt1_{l}")
                g2s = parpool.tile([P, G], F32, tag="g2", name=f"g2_{l}")
                bt2s = parpool.tile([P, G], F32, tag="bt2", name=f"bt2_{l}")
                for t_, d_ in ((g1s, g1_d), (bt1s, bt1_d), (g2s, g2_d), (bt2s, bt2_d)):
                    nc.sync.dma_start(t_, d_[l].rearrange("(o p) -> p o", p=P))
                b1c = parpool.tile([P, GF], F32, tag="b1c", name=f"b1c_{l}")
                nc.sync.dma_start(b1c, b1_d[l].rearrange("(o p) -> p o", p=P))
                b2r = parpool.tile([1, D], CDT, tag="b2r", name=f"b2r_{l}")
                nc.sync.dma_start(b2r, b2_d[l:l + 1, :])

                pair_tiles = {}

                def layernorm_pair(p, r, r2, gam, bet):
                    """LN over features of r [P, G, PT]; writes into
                    xT_s[:, :, pair p]. Emitted directly (DVE/Pool/PE)."""
                    ts = slice(p * PT, (p + 1) * PT)
                    nc.vector.tensor_tensor(r2[:], r[:], r[:], OP.mult)
                    stA = pppool.tile([1, PT], F32, tag="pp",
                                      name=f"stA_{l}_{p}_{id(r)}")
                    stB = pppool.tile([1, PT], F32, tag="pp",
                                      name=f"stB_{l}_{p}_{id(r)}")
                    for gi in range(G):
                        nc.tensor.matmul(stA, ones1, r[:, gi, :],
                                         start=(gi == 0), stop=(gi == G - 1))
                        nc.tensor.matmul(stB, ones1, r2[:, gi, :],
                                         start=(gi == 0), stop=(gi == G - 1))
                    ms = small.tile([1, 4, PT], F32, tag="ms", name=f"ms_{l}_{p}_{id(r)}")
                    nc.vector.tensor_scalar_mul(ms[0:1, 0, :], stA, 1.0 / D)
                    nc.vector.tensor_scalar_mul(ms[0:1, 3, :], stB, 1.0 / D)
                    # var = ex2 - mu^2 -> slot 2
                    nc.vector.tensor_tensor(ms[0:1, 2, :], ms[0:1, 0, :],
                                            ms[0:1, 0, :], OP.mult)
                    nc.vector.tensor_tensor(ms[0:1, 2, :], ms[0:1, 3, :],
                                            ms[0:1, 2, :], OP.subtract)
                    nc.vector.tensor_scalar(out=ms[0:1, 3, :], in0=ms[0:1, 2, :],
                                            scalar1=1.0, scalar2=EPS,
                                            op0=OP.mult, op1=OP.add)
                    quake_rsqrt(ms[0:1, 1, :], ms[0:1, 2, :], ms[0:1, 3, :])
                    bc = small.tile([P, 2, PT], F32, tag="bc", name=f"bc_{l}_{p}_{id(r)}")
                    nc.gpsimd.partition_broadcast(bc, ms[0:1, 0:2, :])
                    mu_b = bc[:, 0:1, :].to_broadcast((P, G, PT))
                    sd_b = bc[:, 1:2, :].to_broadcast((P, G, PT))
                    nc.vector.tensor_tensor(r[:], r[:], mu_b, OP.subtract)
                    nc.vector.tensor_tensor(r[:], r[:], sd_b, OP.mult)
                    gam_b = gam[:, :, None].to_broadcast((P, G, PT))
                    bet_b = bet[:, :, None].to_broadcast((P, G, PT))
                    nc.vector.tensor_tensor(r[:], r[:], gam_b, OP.mult)
                    nc.vector.tensor_tensor(xT_s[:, :, ts], r[:], bet_b, OP.add)

                def proj_pieces(p):
                    """q/k/v projections for pair p. Returns pieces."""
                    ts = slice(p * PT, (p + 1) * PT)
                    qT = qkvpool.tile([P, G, PT], CDT, tag="qT", name=f"qT_{l}_{p}")
                    kT = qkvpool.tile([P, G, PT], CDT, tag="kT", name=f"kT_{l}_{p}")
                    ksx = qkvpool.tile([P, G, PT], CDT, tag="ksx", name=f"ksx_{l}_{p}")
                    v65a = qkvpool.tile([P, 2, 8, 65], CDT, tag="v65a",
                                        name=f"v65_{l}_{2 * p}")
                    v65b = qkvpool.tile([P, 2, 8, 65], CDT, tag="v65b",
                                        name=f"v65_{l}_{2 * p + 1}")
                    pair_tiles[p] = (qT, kT, ksx, v65a, v65b)
                    pieces = []

                    def ones_col(v65):
                        nc.vector.tensor_copy(out=v65[:, :, :, 64:65], in_=ones_c)
                    pieces.append(lambda: ones_col(v65a))
                    pieces.append(lambda: ones_col(v65b))

                    # q/k chains: for each g, 4-ki accumulation; interleave
                    # q-chain with k-chain so banks alternate.
                    def qk_mm(pq, pk, g, ki):
                        nc.tensor.matmul(pq, wa[:, ki, QO + g * P:QO + (g + 1) * P],
                                         xT_s[:, ki, ts],
                                         start=(ki == 0), stop=(ki == G - 1))
                        nc.tensor.matmul(pk, wa[:, ki, KO + g * P:KO + (g + 1) * P],
                                         xT_s[:, ki, ts],
                                         start=(ki == 0), stop=(ki == G - 1))

                    def qk_copy(pq, pk, g):
                        nc.vector.tensor_copy(out=qT[:, g, :], in_=pq)
                        nc.vector.tensor_copy(out=kT[:, g, :], in_=pk)
                        nc.sync.dma_start(ksx[0:64, g, :], kT[64:128, g, :])
                        nc.sync.dma_start(ksx[64:128, g, :], kT[0:64, g, :])

                    for g in range(G):
                        pq = pppool.tile([P, PT], F32, tag="pp", name=f"pq_{l}_{p}_{g}")
                        pk = pppool.tile([P, PT], F32, tag="pp", name=f"pk_{l}_{p}_{g}")
                        for ki in range(G):
                            pieces.append(lambda pq=pq, pk=pk, g=g, ki=ki:
                                          qk_mm(pq, pk, g, ki))
                        pieces.append(lambda pq=pq, pk=pk, g=g: qk_copy(pq, pk, g))

                    # v projection per slab: out pv [128 tok, 512 feat]
                    def v_mm(pv1, pv2, h, ki):
                        hs = h * T
                        nc.tensor.matmul(pv1, xT_s[:, ki, hs:hs + P],
                                         wa[:, ki, VO:VO + D],
                                         start=(ki == 0), stop=(ki == G - 1))
                        nc.tensor.matmul(pv2, xT_s[:, ki, hs + P:hs + 2 * P],
                                         wa[:, ki, VO:VO + D],
                                         start=(ki == 0), stop=(ki == G - 1))

                    def v_copy(pv1, pv2, v65):
                        nc.vector.tensor_copy(out=v65[:, 0, :, 0:64], in_=pv1)
                        nc.vector.tensor_copy(out=v65[:, 1, :, 0:64], in_=pv2)

                    for ci, v65 in ((0, v65a), (1, v65b)):
                        h = 2 * p + ci
                        pv1 = pppool.tile([P, D], F32, tag="pp", name=f"pv1_{l}_{h}")
                        pv2 = pppool.tile([P, D], F32, tag="pp", name=f"pv2_{l}_{h}")
                        for ki in range(G):
                            pieces.append(lambda pv1=pv1, pv2=pv2, h=h, ki=ki:
                                          v_mm(pv1, pv2, h, ki))
                        pieces.append(lambda pv1=pv1, pv2=pv2, v65=v65:
                                      v_copy(pv1, pv2, v65))
                    return pieces

                def attn(h, steal):
                    """Attention for slab h; pops steal pieces to fill PE."""
                    p, c = h // 2, h % 2
                    qT, kT, ksx, v65a, v65b = pair_tiles[p]
                    v65 = v65a if c == 0 else v65b
                    if c == 0:
                        oT = oTpool.tile([P, G, PT], CDT, tag="oT", name=f"oT_{l}_{p}")
                        pair_tiles[(p, "oT")] = oT
                    else:
                        oT = pair_tiles[(p, "oT")]
                    cs = slice(c * T, (c + 1) * T)

                    for sh in range(2):  # m-halves {0,1}, {2,3}
                        poE = popool.tile([65, 512], F32, tag="poE",
                                          name=f"poE_{l}_{h}_{sh}")
                        poO = popool.tile([65, 512], F32, tag="poO",
                                          name=f"poO_{l}_{h}_{sh}")
                        rhsE = qT[0:64, 2 * sh:2 * sh + 2, cs]
                        rhsO = qT[64:128, 2 * sh:2 * sh + 2, cs]
                        prev = None
                        for t in range(16):  # t-chunk = (jb, cc)
                            jb, cc = t // 2, t % 2
                            um = umpool.tile([P, 1024], F32, tag="um",
                                             name=f"um_{l}_{h}_{sh}_{t}")
                            kcol = slice(c * T + cc * P, c * T + (cc + 1) * P)
                            if jb % 2 == 0:
                                lhsE = kT[0:64, jb // 2, kcol]
                                lhsO = ksx[64:128, jb // 2, kcol]
                            else:
                                lhsE = ksx[0:64, jb // 2, kcol]
                                lhsO = kT[64:128, jb // 2, kcol]
                            nc.tensor.matmul(um[:, 0:512], lhsE, rhsE,
                                             start=True, stop=True)
                            nc.tensor.matmul(um[:, 512:1024], lhsO, rhsO,
                                             start=True, stop=True)
                            pX = pXpool.tile([P, 1024], CDT, tag="pX",
                                             name=f"pX_{l}_{h}_{sh}_{t}")
                            nc.scalar.activation(out=pX, in_=um, func=AF.Exp,
                                                 scale=0.125)
                            if prev is not None:
                                pXp, tp = prev
                                jbp, ccp = tp // 2, tp % 2
                                nc.tensor.matmul(poE, v65[:, ccp, jbp, :],
                                                 pXp[:, 0:512],
                                                 start=(tp == 0), stop=False)
                                nc.tensor.matmul(poO, v65[:, ccp, jbp, :],
                                                 pXp[:, 512:1024],
                                                 start=(tp == 0), stop=False)
                            prev = (pX, t)
                            if steal:
                                steal.popleft()()
                        pXp, tp = prev
                        jbp, ccp = tp // 2, tp % 2
                        nc.tensor.matmul(poE, v65[:, ccp, jbp, :], pXp[:, 0:512],
                                         start=False, stop=True)
                        nc.tensor.matmul(poO, v65[:, ccp, jbp, :], pXp[:, 512:1024],
                                         start=False, stop=True)

                        # denominators -> oT (normalized, feature-major)
                        rec = small.tile([1, 1024], F32, tag="rec",
                                         name=f"rec_{l}_{h}_{sh}")
                        nc.vector.reciprocal(out=rec[:, 0:512], in_=poE[64:65, :])
                        nc.vector.reciprocal(out=rec[:, 512:1024], in_=poO[64:65, :])
                        bcd = small.tile([64, 1024], F32, tag="bcd",
                                         name=f"bcd_{l}_{h}_{sh}")
                        nc.gpsimd.partition_broadcast(bcd, rec)
                        for mi in range(2):
                            m = 2 * sh + mi
                            nc.vector.tensor_tensor(
                                oT[0:64, m, cs], poE[0:64, mi * T:(mi + 1) * T],
                                bcd[0:64, mi * T:(mi + 1) * T], OP.mult)
                            nc.vector.tensor_tensor(
                                oT[64:128, m, cs], poO[0:64, mi * T:(mi + 1) * T],
                                bcd[0:64, 512 + mi * T:512 + (mi + 1) * T], OP.mult)

                def wo_ln1(p):
                    """wo projection + residual + LN1 for pair p (direct)."""
                    ts = slice(p * PT, (p + 1) * PT)
                    oT = pair_tiles[(p, "oT")]
                    r = rpool.tile([P, G, PT], CDT, tag="r", name=f"r1_{l}_{p}")
                    r2 = rpool.tile([P, G, PT], CDT, tag="r2", name=f"r12_{l}_{p}")
                    for gpair in range(2):
                        pyA = pppool.tile([P, PT], F32, tag="pp",
                                          name=f"wo{l}_{p}_{gpair}a")
                        pyB = pppool.tile([P, PT], F32, tag="pp",
                                          name=f"wo{l}_{p}_{gpair}b")
                        gA, gB = 2 * gpair, 2 * gpair + 1
                        for gi in range(G):
                            nc.tensor.matmul(pyA, wa[:, gi, OO + gA * P:OO + (gA + 1) * P],
                                             oT[:, gi, :],
                                             start=(gi == 0), stop=(gi == G - 1))
                            nc.tensor.matmul(pyB, wa[:, gi, OO + gB * P:OO + (gB + 1) * P],
                                             oT[:, gi, :],
                                             start=(gi == 0), stop=(gi == G - 1))
                        nc.vector.tensor_tensor(r[:, gA, :], pyA, xT_s[:, gA, ts], OP.add)
                        nc.vector.tensor_tensor(r[:, gB, :], pyB, xT_s[:, gB, ts], OP.add)
                    layernorm_pair(p, r, r2, g1s, bt1s)

                def ffn1_pieces(p):
                    ts = slice(p * PT, (p + 1) * PT)
                    fT = fTpool.tile([P, GF, PT], CDT, tag="fT", name=f"fT_{l}_{p}")
                    pair_tiles[(p, "fT")] = fT
                    pieces = []

                    def mm(ppA, ppB, dkA, dkB, ki):
                        nc.tensor.matmul(ppA, w1s[:, ki, dkA * P:(dkA + 1) * P],
                                         xT_s[:, ki, ts],
                                         start=(ki == 0), stop=(ki == G - 1))
                        nc.tensor.matmul(ppB, w1s[:, ki, dkB * P:(dkB + 1) * P],
                                         xT_s[:, ki, ts],
                                         start=(ki == 0), stop=(ki == G - 1))

                    def relu(ppA, ppB, dkA, dkB):
                        nc.scalar.activation(out=fT[:, dkA, :], in_=ppA, func=AF.Relu,
                                             bias=b1c[:, dkA:dkA + 1], scale=1.0)
                        nc.scalar.activation(out=fT[:, dkB, :], in_=ppB, func=AF.Relu,
                                             bias=b1c[:, dkB:dkB + 1], scale=1.0)

                    for dk in range(0, GF, 2):
                        ppA = pppool.tile([P, PT], F32, tag="pp", name=f"f1a_{l}_{p}_{dk}")
                        ppB = pppool.tile([P, PT], F32, tag="pp", name=f"f1b_{l}_{p}_{dk}")
                        for ki in range(G):
                            pieces.append(lambda ppA=ppA, ppB=ppB, dk=dk, ki=ki:
                                          mm(ppA, ppB, dk, dk + 1, ki))
                        pieces.append(lambda ppA=ppA, ppB=ppB, dk=dk:
                                      relu(ppA, ppB, dk, dk + 1))
                    return pieces

                def ffn2_pieces(p):
                    ts = slice(p * PT, (p + 1) * PT)
                    fT = pair_tiles[(p, "fT")]
                    r = rpool.tile([P, G, PT], CDT, tag="r", name=f"r2a_{l}_{p}")
                    r2 = rpool.tile([P, G, PT], CDT, tag="r2", name=f"r2b_{l}_{p}")
                    pieces = []

                    def bias_mm(ppA, ppB, gA, gB):
                        nc.tensor.matmul(ppA, b2r[0:1, gA * P:(gA + 1) * P], ones_row,
                                         start=True, stop=False)
                        nc.tensor.matmul(ppB, b2r[0:1, gB * P:(gB + 1) * P], ones_row,
                                         start=True, stop=False)

                    def mm(ppA, ppB, gA, gB, fi):
                        nc.tensor.matmul(ppA, w2s[:, fi, gA * P:(gA + 1) * P],
                                         fT[:, fi, :],
                                         start=False, stop=(fi == GF - 1))
                        nc.tensor.matmul(ppB, w2s[:, fi, gB * P:(gB + 1) * P],
                                         fT[:, fi, :],
                                         start=False, stop=(fi == GF - 1))

                    def res(ppA, ppB, gA, gB):
                        nc.vector.tensor_tensor(r[:, gA, :], ppA, xT_s[:, gA, ts], OP.add)
                        nc.vector.tensor_tensor(r[:, gB, :], ppB, xT_s[:, gB, ts], OP.add)

                    for gpair in range(2):
                        gA, gB = 2 * gpair, 2 * gpair + 1
                        ppA = pppool.tile([P, PT], F32, tag="pp", name=f"f2a_{l}_{p}_{gpair}")
                        ppB = pppool.tile([P, PT], F32, tag="pp", name=f"f2b_{l}_{p}_{gpair}")
                        pieces.append(lambda ppA=ppA, ppB=ppB, gA=gA, gB=gB:
                                      bias_mm(ppA, ppB, gA, gB))
                        for fi in range(0, GF, 2):
                            pieces.append(lambda ppA=ppA, ppB=ppB, gA=gA, gB=gB, fi=fi:
                                          (mm(ppA, ppB, gA, gB, fi),
                                           mm(ppA, ppB, gA, gB, fi + 1)))
                        pieces.append(lambda ppA=ppA, ppB=ppB, gA=gA, gB=gB:
                                      res(ppA, ppB, gA, gB))
                    pieces.append(lambda: layernorm_pair(p, r, r2, g2s, bt2s))
                    return pieces

                # ---------------- pipeline ----------------
                steal = deque()
                for piece in proj_pieces(0):
                    piece()
                for p in range(n_pairs):
                    if p + 1 < n_pairs:
                        steal.extend(proj_pieces(p + 1))
                    if p >= 1:
                        steal.extend(ffn1_pieces(p - 1))
                        steal.extend(ffn2_pieces(p - 1))
                    attn(2 * p, steal)
                    attn(2 * p + 1, steal)
                    wo_ln1(p)
                while steal:
                    steal.popleft()()
                for piece in ffn1_pieces(n_pairs - 1):
                    piece()
                for piece in ffn2_pieces(n_pairs - 1):
                    piece()

            for g in range(G):
                nc.sync.dma_start(out_d[g * P:(g + 1) * P, :], xT_s[:, g, :])

    nc.compile()
    return nc


def _get_nc(n_layers=N_LAYERS, n_pairs=NPAIR):
    key = (n_layers, n_pairs)
    if key not in _CACHE:
        _CACHE[key] = _build(n_layers, n_pairs)
    return _CACHE[key]


def make_in_maps(x, wq, wk, wv, wo, w1, b1, w2, b2, g1, beta1, g2, beta2):
    import ml_dtypes
    BF = ml_dtypes.bfloat16
    x = np.asarray(x, np.float32)
    common = {
        "wq": np.ascontiguousarray(np.asarray(wq, np.float32).astype(BF)),
        "wk": np.ascontiguousarray(np.asarray(wk, np.float32).astype(BF)),
        "wv": np.ascontiguousarray(np.asarray(wv, np.float32).astype(BF)),
        "wo": np.ascontiguousarray(np.asarray(wo, np.float32).astype(BF)),
        "w1": np.ascontiguousarray(np.asarray(w1, np.float32).astype(BF)),
        "w2": np.ascontiguousarray(np.asarray(w2, np.float32).astype(BF)),
        "b1": np.ascontiguousarray(np.asarray(b1, np.float32)),
        "b2": np.ascontiguousarray(np.asarray(b2, np.float32).astype(BF)),
        "g1": np.ascontiguousarray(np.asarray(g1, np.float32)),
        "beta1": np.ascontiguousarray(np.asarray(beta1, np.float32)),
        "g2": np.ascontiguousarray(np.asarray(g2, np.float32)),
        "beta2": np.ascontiguousarray(np.asarray(beta2, np.float32)),
    }
    return [{"xT": np.ascontiguousarray(x[b].T.astype(BF)), **common}
            for b in range(B)]


def kernel(x, wq, wk, wv, wo, w1, b1, w2, b2, g1, beta1, g2, beta2,
           _n_layers=N_LAYERS, _trace=False):
    from concourse.bass_utils import run_bass_kernel_spmd

    nc = _get_nc(_n_layers)
    in_maps = make_in_maps(x, wq, wk, wv, wo, w1, b1, w2, b2,
                           g1, beta1, g2, beta2)
    res = run_bass_kernel_spmd(nc, in_maps, core_ids=list(range(B)), trace=_trace)
    out = np.stack([np.asarray(res.results[b]["out"]).astype(np.float32).T
                    for b in range(B)])
    if _trace:
        kernel.last_exec_time_ns = res.exec_time_ns
        kernel.last_results = res
    return out.astype(np.float32)
